# revision 35
# baseline (speedup 1.0000x reference)
"""GCN forward (gather + segment-sum + matmul) on 8 TRN2 NeuronCores.

Algorithm (factorized GCN):
    out[i] = deg[i] * (sum_{j in N(i)} deg[j] * X[j]) @ W

Sharding: destination nodes are split across the 8 cores (12500 rows each);
the deg_src-prescaled fp16 feature table X' = deg[:,None]*X is replicated to
every core's HBM. Each core:
  - bin-packs its 12500 dests into 200 windows of <=64 (64-wide one-hots
    halve the DVE sel build and the per-edge-tile matmul stream vs 128;
    window pairs share one 128-slot output block), every (chunk,window)
    cell holds <=256 edges = exactly 2 gather tiles (~1.4% tile padding),
  - gathers the fp16 rows of X' for its ~200K edges with gpsimd dma_gather
    on 4 SWDGE queues (the bottleneck: Q7 desc-gen is ~90% busy and the HW
    activity throttle holds utilization near 0.5; int16 gather indices
    force a 4-way chunking of the 100K-row table, so each core keeps 4
    chunk-local edge streams, each cell's edges sorted by source),
  - the 4 streams are padded to EQUAL tile counts and calls are emitted in
    strict q0->q3 rotation with all first-call indices landed by one DMA:
    the Tile scheduler assigns SWDGE completion sems round-robin over 8
    lanes ignoring queue_num, and only a periodic issue order keeps each
    lane locked to one queue (violations corrupt gather completions),
  - builds one-hot matrices sel[e,d] = (dstrel[e] == d) in batches of SB_T
    tiles with a single broadcast-AP DVE is_equal,
  - segment-sums via TensorE: A_T[f,d] += G[e,f]^T @ sel[e,d], accumulating
    in PSUM over a window's edge tiles round-robined across the 4 chunk
    streams; window pairs then share the @W matmul and deg_dest scaling,
  - flushes eighths of the persistent SBUF output buffer as they finish,
  - the host inverse-permutes the rows (window packing) and concatenates.

The per-edge aggregation, both matmuls and the deg_dest scaling run on
device; the host computes indices/partitioning and stages dtype-converted,
deg_src-prescaled inputs.
"""
import os

import numpy as np

N = 100000
E = 1600000
F = 128
P = 128
NCORES = 8
NPC = N // NCORES          # 12500 destination rows per core
# Destination windows are W=64 wide (halving the one-hot sel width and the
# per-edge-tile matmul stream from 128 to 64 columns cuts DVE and TensorE
# activity ~2x, easing the HW activity throttle). Two windows pair up into
# one 128-slot output block so the W@weight matmul / outbuf layout keeps
# full 128-partition width.
W = 64
NW = 200
NPAIR = NW // 2
NQ = 4                     # table chunks (int16 gather indices)
CHUNK = 25000              # rows per chunk
# Tiles per gather call. Per-call num_idxs is capped by the SWDGE
# descriptor-ring carveout, which scales with dynamic_dma_scratch_size:
# ~97 descs/lane at the 16KB default (13 tiles crashed the device), ~194
# at 32KB (20 tiles = 161 descs/lane verified on HW). Fewer, bigger calls
# amortize the ~1us fixed Q7 desc-gen cost per call.
_gbt_env = os.environ.get("GCN_GB_TILES", "20")
GBT = [int(x) for x in (_gbt_env.split(",") * 4)[:4]] if "," in _gbt_env \
    else [int(_gbt_env)] * 4
GB_TILES = max(GBT)

_PROGRAM_CACHE: dict = {}


def _row_ids_from_pointers(row_pointers: np.ndarray) -> np.ndarray:
    """Replicates jnp.repeat(arange(N), diff(rp), total_repeat_length=E)."""
    rl = np.diff(row_pointers.astype(np.int64))
    starts = np.concatenate([np.zeros(1, np.int64), np.cumsum(rl)[:-1]])
    return np.searchsorted(starts, np.arange(E, dtype=np.int64), side="right") - 1


def _group_dests(cnt):
    """Bin-pack one core's destinations into NW windows of <=W dests.

    cnt: [NPC, NQ] per-dest per-chunk edge counts. Windows 2..NW-1 are
    hard-capped at 2*P edges per chunk (exactly 2 gather tiles); windows
    0-1 absorb the overflow. Any grouping is correct (t_qw is computed
    from actual counts); the caps only minimize tile padding.
    Returns (grp [NPC], pos [NPC]).
    """
    CAP = 4 * P * W // 128
    NREG = NW - 2
    loads = np.zeros((NREG, NQ), np.int64)
    sizes = np.zeros(NREG, np.int64)
    grp = np.full(NPC, -1, np.int64)
    order = np.argsort(-cnt.max(axis=1), kind="stable")
    overflow = []
    big = 1 << 40
    for d in order:
        v = cnt[d]
        cand = loads + v
        score = cand.max(axis=1)
        score[(cand > CAP).any(axis=1) | (sizes >= W)] = big
        g = int(np.argmin(score))
        if score[g] >= big:
            overflow.append(d)
            continue
        loads[g] = cand[g]
        sizes[g] += 1
        grp[d] = g + 2
    # overflow windows 0/1: size-capped only
    osz = [0, 0]
    for d in overflow:
        g = 0 if osz[0] <= osz[1] and osz[0] < W else 1
        if osz[g] >= W:  # both full: spill into least-loaded regular window
            g2 = int(np.argmin(np.where(sizes < W, loads.max(axis=1), big)))
            loads[g2] += cnt[d]
            sizes[g2] += 1
            grp[d] = g2 + 2
            continue
        osz[g] += 1
        grp[d] = g
    # positions within each window
    pos = np.zeros(NPC, np.int64)
    nxt = np.zeros(NW, np.int64)
    for d in range(NPC):
        g = grp[d]
        pos[d] = nxt[g]
        nxt[g] += 1
    return grp, pos


def _preprocess(X, weight, degrees, row_pointers, column_index):
    row_ids = _row_ids_from_pointers(row_pointers)          # [E] sorted, in [0,N)
    col = column_index.astype(np.int64)
    deg = np.ascontiguousarray(degrees.astype(np.float32))

    core = row_ids // NPC                                   # [E] in [0,8)
    local = row_ids - core * NPC
    q = col // CHUNK                                        # [E] in [0,4)
    src16_all = (col - q * CHUNK).astype(np.int16)

    # balanced dest->window grouping per core (kills tile padding)
    w_local = np.empty(E, np.int64)
    dstrel_all = np.empty(E, np.float32)
    dest_of = np.full((NCORES, NW * W), -1, np.int64)       # (c, w*W+p) -> node id
    for c in range(NCORES):
        m = core == c
        cnt = np.bincount(local[m] * NQ + q[m], minlength=NPC * NQ).reshape(NPC, NQ)
        grp, pos = _group_dests(cnt)
        w_local[m] = grp[local[m]]
        dstrel_all[m] = pos[local[m]]
        dest_of[c, grp * W + pos] = c * NPC + np.arange(NPC)

    key = ((core * NQ + q) * NW + w_local).astype(np.int64)  # (c, q, w)
    counts = np.bincount(key, minlength=NCORES * NQ * NW).reshape(NCORES, NQ, NW)
    t_qw = -(-counts.max(axis=0) // P)                       # [NQ, NW]
    # Equalize the 4 chunk streams to the same tile count (pad tiles have
    # src=0, dstrel=-1 and contribute nothing): equal streams mean every
    # queue issues the same number of gather calls, so the strict
    # q0->q3 call rotation in the device program stays exactly periodic.
    # The Tile scheduler assigns SWDGE completion sems round-robin over 8
    # lanes ignoring queue_num; only a periodic issue order keeps each
    # lane pinned to one queue (cross-queue increments desync the ucode's
    # per-queue reclaim and corrupt the gather data).
    T_tiles = int(t_qw.sum(axis=1).max())
    for qq in range(NQ):
        t_qw[qq, 0] += T_tiles - int(t_qw[qq].sum())
    lq = t_qw.sum(axis=1) * P                                # [NQ] stream lengths
    chunk_base = np.concatenate([np.zeros(1, np.int64), np.cumsum(lq)])
    ltot = int(chunk_base[-1])
    # offset of window w's padded segment within chunk q's stream
    offs_qw = np.cumsum(np.concatenate([np.zeros((NQ, 1), np.int64), t_qw[:, :-1]], axis=1) * P, axis=1) \
        if False else (np.cumsum(t_qw, axis=1) - t_qw) * P   # [NQ, NW] exclusive prefix

    # within each (core,chunk,window) cell, order edges by ascending source so
    # each SDMA engine's gather descriptors walk HBM mostly monotonically
    order = np.lexsort((src16_all, key))
    key_s = key[order]
    starts_flat = np.concatenate([np.zeros(1, np.int64), np.cumsum(counts.reshape(-1))])[:-1]
    rank_s = np.arange(E, dtype=np.int64) - starts_flat[key_s]
    q_s = (key_s // NW) % NQ
    w_s = key_s % NW
    core_s = key_s // (NQ * NW)
    pos_s = chunk_base[q_s] + offs_qw[q_s, w_s] + rank_s     # [E] position in core's array

    src_pad = np.zeros((NCORES, ltot), np.int16)
    dstrel_pad = np.full((NCORES, ltot), -1.0, np.float32)
    src_pad[core_s, pos_s] = src16_all[order]
    dstrel_pad[core_s, pos_s] = dstrel_all[order]

    # per-chunk device layouts
    idx_w, dst_t = [], []
    for qq in range(NQ):
        sl = slice(int(chunk_base[qq]), int(chunk_base[qq + 1]))
        s = src_pad[:, sl]                                   # [NC, LQ]
        # wrapped idx layout [128, LQ/16]: idx i at [i%16, i//16], replicated 8x
        iw = np.tile(s.reshape(NCORES, -1, 16).transpose(0, 2, 1), (1, 8, 1))
        idx_w.append(np.ascontiguousarray(iw))
        dst_t.append(np.ascontiguousarray(
            dstrel_pad[:, sl].reshape(NCORES, -1, P).transpose(0, 2, 1).astype(np.float16)))

    # per-core dest-degree table [P, NPAIR]: window pair pw = w//2 holds
    # window w's dests at partition s = (w%2)*W + pos
    degt = np.zeros((NCORES, P, NPAIR), np.float32)
    for c in range(NCORES):
        ids = dest_of[c]                                     # [NW*W]
        dv = np.where(ids >= 0, deg[np.clip(ids, 0, N - 1)], 0.0)
        degt[c] = dv.reshape(NPAIR, 2, W).transpose(1, 2, 0).reshape(P, NPAIR)

    # stage deg_src-prescaled features: the weighted segment-sum's per-edge
    # weights deg[col] fold into the gathered rows (host staging, like the
    # dtype conversion); the aggregation itself stays on device
    xt = np.ascontiguousarray((X.astype(np.float32) * deg[:, None]).astype(np.float16))
    w16 = np.ascontiguousarray(weight.astype(np.float16))
    t_key = tuple(tuple(int(x) for x in row) for row in t_qw)
    return xt, w16, idx_w, dst_t, degt, dest_of, t_key


SB_T = int(os.environ.get("GCN_SB_T", "8"))  # tiles per batched sel build


def _build_program(t_qw):
    import concourse.bacc as bacc
    import concourse.bass as bass
    import concourse.mybir as mybir
    import concourse.tile as tile

    lq = [sum(t_qw[q]) * P for q in range(NQ)]

    scratch = int(os.environ.get("GCN_SCRATCH", "32768"))
    nc = bacc.Bacc("TRN2", target_bir_lowering=False, num_swdge_queues=4,
                   dynamic_dma_scratch_size=scratch)
    xt_p = nc.declare_dram_parameter("xt", [N, F], mybir.dt.float16, isOutput=False)
    # All 4 queues' gather indices ride TWO shared parameters, each loaded
    # by ONE DMA: a small "first block" slab (gates every queue's first
    # gather, lands in ~1us) and the rest. One DMA per slab keeps every
    # queue's next call becoming ready at the same instant, so the tile
    # scheduler's issue order (and thus its round-robin DMASW lane
    # assignment) stays in strict q0->q3 rotation; staggered per-queue
    # loads would let q0 run ahead and cross-queue lane locks corrupt the
    # gather completion tracking.
    idx_step = lq[0] // 16
    assert all(l // 16 == idx_step for l in lq)
    g0cols = min(GBT[0] * P // 16, idx_step)
    idx0_p = nc.declare_dram_parameter("idx0blk", [P, NQ * g0cols], mybir.dt.int16, isOutput=False)
    idxrest_p = nc.declare_dram_parameter("idxrest", [P, NQ * (idx_step - g0cols)], mybir.dt.int16, isOutput=False) \
        if idx_step > g0cols else None
    dst_ps = [nc.declare_dram_parameter(f"dstrel{q}", [P, lq[q] // P], mybir.dt.float16, isOutput=False) for q in range(NQ)]
    degt_p = nc.declare_dram_parameter("degt", [P, NPAIR], mybir.dt.float32, isOutput=False)
    w_p = nc.declare_dram_parameter("w16", [F, F], mybir.dt.float16, isOutput=False)
    # transposed output layout: row s holds pair-major features so the
    # final DMA is one contiguous 49KB-per-partition write (host unshuffles)
    out_p = nc.declare_dram_parameter("out", [P, NPAIR * F], mybir.dt.float32, isOutput=True)

    def bcast_mid(ap, t):
        # [128, t] AP -> [128, t, W] with stride-0 inner (value per (p, tile))
        return bass.AP(ap.tensor, ap.offset, [ap.ap[0], [ap.ap[1][0], t], [0, W]])

    # prep/trigger split measured 5x SLOWER (per-call trigger+sem overhead in
    # Tile mode swamps the gen/drain overlap it buys); keep the blocking form
    use_prep = os.environ.get("GCN_PREP", "0") == "1"
    dma_sems = [nc.alloc_semaphore(f"swdge_dma{q}") for q in range(NQ)] if use_prep else None
    with tile.TileContext(nc) as tc:
        with (
            tc.tile_pool(name="persist", bufs=1) as persist,
            tc.tile_pool(name="gblk", bufs=int(os.environ.get("GCN_GBUFS", "2"))) as gpool,
            tc.tile_pool(name="selp", bufs=int(os.environ.get("GCN_SBUFS", "2"))) as selpool,
            tc.tile_pool(name="atsb", bufs=2) as atpool,
            tc.tile_pool(name="outsb", bufs=2) as outpool,
            tc.tile_pool(name="psum1", bufs=2, space="PSUM") as psum1,
            tc.tile_pool(name="psum2", bufs=2, space="PSUM") as psum2,
        ):
            # (Tried issuing the small loads on the Scalar HWDGE ring so
            # they don't queue behind the 3MB idxrest transfer: the first
            # gather only moved 24.5->22.8us — the ~21us head is a
            # scheduler stage-reset barrier waiting on ALL stage-0 DMAs
            # including idxrest, so the ring choice is neutral.)
            idx0_sb = persist.tile([P, NQ * g0cols], mybir.dt.int16, tag="idx0blk", name="idx0blk")
            nc.sync.dma_start(idx0_sb[:], idx0_p[:])
            rest_cols = idx_step - g0cols
            if idxrest_p is not None:
                idxr_sb = persist.tile([P, NQ * rest_cols], mybir.dt.int16, tag="idxrest", name="idxrest")
                nc.sync.dma_start(idxr_sb[:], idxrest_p[:])
            dst_sb = []
            for q in range(NQ):
                t2 = persist.tile([P, lq[q] // P], mybir.dt.float16, tag=f"dst{q}", name=f"dst{q}")
                nc.sync.dma_start(t2[:], dst_ps[q][:])
                dst_sb.append(t2)
            degt_sb = persist.tile([P, NPAIR], mybir.dt.float32)
            nc.sync.dma_start(degt_sb[:], degt_p[:])
            w_sb = persist.tile([F, F], mybir.dt.float16)
            nc.sync.dma_start(w_sb[:], w_p[:])
            c_i32 = persist.tile([P, P], mybir.dt.int32)
            nc.gpsimd.iota(c_i32[:], pattern=[[1, P]], base=0, channel_multiplier=0)
            c_f16 = persist.tile([P, P], mybir.dt.float16)
            nc.vector.tensor_copy(c_f16[:], c_i32[:])

            outbuf = persist.tile([P, NPAIR * F], mybir.dt.float32, name="outbuf")

            pos = [0] * NQ
            calls_done = [0] * NQ
            gblk = [None] * NQ
            selblk = [None] * NQ
            # strict round-robin gather-call emission: all 4 streams have
            # equal tile counts (t_qw equalized in _preprocess) and every
            # call is GBT tiles, so the SWDGE instruction order is exactly
            # periodic q0,q1,q2,q3 and the scheduler's round-robin DMASW
            # lane assignment keeps each lane locked to a single queue.
            T_tiles = lq[0] // P
            assert all(l // P == T_tiles for l in lq)
            GBT_U = GBT[0]
            covered = [0] * NQ           # tiles covered by emitted calls
            blocks = [[] for _ in range(NQ)]   # FIFO of (start, end, tile, call#)
            cur_start = [0] * NQ
            cur_end = [0] * NQ
            cur_call = [0] * NQ
            rot = [0]

            def emit_one(qe):
                start = covered[qe]
                nt_call = min(GBT_U, T_tiles - start)
                if nt_call <= 0:
                    return
                nidx = nt_call * P
                t = gpool.tile([P, GBT_U * F], mybir.dt.float16,
                               tag=f"gblk{qe}", name=f"gblk{qe}")
                c0 = start * P // 16
                c1 = (start * P + nidx) // 16
                if start == 0:
                    assert c1 <= g0cols
                    iap = idx0_sb[:, qe * g0cols : qe * g0cols + c1]
                else:
                    iap = idxr_sb[:, qe * rest_cols + (c0 - g0cols)
                                  : qe * rest_cols + (c1 - g0cols)]
                gather_kw = dict(
                    out_ap=t[:, : nt_call * F].rearrange("p (k f) -> p k f", f=F),
                    in_ap=xt_p[qe * CHUNK : (qe + 1) * CHUNK, :],
                    idxs_ap=iap,
                    num_idxs=nidx,
                    num_idxs_reg=nidx,
                    elem_size=F,
                    queue_num=qe,
                    single_packet=(os.environ.get('GCN_SP', '0') == '1'),
                )
                if use_prep:
                    nc.gpsimd.dma_gather(
                        prepare_only=True, sem=dma_sems[qe], **gather_kw
                    )
                    nc.gpsimd.trigger_dma(count=None, queue_num=qe)
                    calls_done[qe] += 1
                else:
                    nc.gpsimd.dma_gather(**gather_kw)
                blocks[qe].append((start, start + nt_call, t, calls_done[qe]))
                covered[qe] += nt_call

            flushed = 0
            for pw in range(NPAIR):
                wins = (2 * pw, 2 * pw + 1)
                nt_h = [sum(t_qw[q][w] for q in range(NQ)) for w in wins]
                if nt_h[0] + nt_h[1] == 0:
                    nc.vector.memset(outbuf[:, pw * F : (pw + 1) * F], 0.0)
                    continue
                at_sb = atpool.tile([F, P], mybir.dt.float16)
                for half, w in enumerate(wins):
                    if nt_h[half] == 0:
                        nc.vector.memset(at_sb[:, half * W : (half + 1) * W], 0.0)
                        continue
                    at_ps = psum1.tile([F, W], mybir.dt.float32, space="PSUM")
                    k = 0
                    # round-robin the window's tiles across the 4 chunk
                    # streams so the gather buffers free at an even pace
                    rr = [q for t in range(max(t_qw[q][w] for q in range(NQ)))
                          for q in range(NQ) if t < t_qw[q][w]]
                    if os.environ.get("GCN_RR", "1") != "1":
                        rr = [q for q in range(NQ) for _ in range(t_qw[q][w])]
                    for q in rr:
                        while pos[q] >= cur_end[q]:
                            while not blocks[q]:
                                emit_one(rot[0])
                                rot[0] = (rot[0] + 1) % NQ
                            (cur_start[q], cur_end[q], gblk[q],
                             cur_call[q]) = blocks[q].pop(0)
                        if pos[q] % SB_T == 0:
                            nt_s = min(SB_T, lq[q] // P - pos[q])
                            selblk[q] = selpool.tile(
                                [P, SB_T * W], mybir.dt.float16,
                                tag=f"sel{q}", name=f"sel{q}",
                            )
                            c_b = bass.AP(c_f16[:].tensor, c_f16[:].offset,
                                          [c_f16[:].ap[0], [0, nt_s], [1, W]])
                            nc.vector.tensor_tensor(
                                out=selblk[q][:, : nt_s * W].rearrange("p (t w) -> p t w", w=W),
                                in0=c_b,
                                in1=bcast_mid(dst_sb[q][:, pos[q] : pos[q] + nt_s], nt_s),
                                op=mybir.AluOpType.is_equal,
                            )
                        j = pos[q] - cur_start[q]
                        js = pos[q] % SB_T
                        if use_prep and j == 0:
                            # gate the block's first consumer on DMA landed
                            nc.tensor.wait_ge(dma_sems[q], 16 * cur_call[q])
                        nc.tensor.matmul(
                            out=at_ps[:],
                            lhsT=gblk[q][:, j * F : (j + 1) * F],
                            rhs=selblk[q][:, js * W : (js + 1) * W],
                            start=(k == 0),
                            stop=(k == nt_h[half] - 1),
                        )
                        pos[q] += 1
                        k += 1
                    nc.scalar.activation(at_sb[:, half * W : (half + 1) * W],
                                         at_ps[:], mybir.ActivationFunctionType.Copy)
                o2_ps = psum2.tile([P, F], mybir.dt.float32, space="PSUM")
                nc.tensor.matmul(out=o2_ps[:], lhsT=at_sb[:], rhs=w_sb[:], start=True, stop=True)
                nc.scalar.activation(outbuf[:, pw * F : (pw + 1) * F], o2_ps[:],
                                     mybir.ActivationFunctionType.Copy,
                                     scale=degt_sb[:, pw : pw + 1])
                # flush finished eighths of the output buffer so the
                # writeback overlaps compute instead of trailing serially
                if (pw + 1) % (NPAIR // 8) == 0 and pw + 1 < NPAIR:
                    nc.sync.dma_start(out=out_p[:, flushed * F : (pw + 1) * F],
                                      in_=outbuf[:, flushed * F : (pw + 1) * F])
                    flushed = pw + 1
            nc.sync.dma_start(out=out_p[:, flushed * F :], in_=outbuf[:, flushed * F :])
    nc.compile()
    return nc


def _get_program(t_key):
    key = (t_key, tuple(GBT), SB_T, os.environ.get("GCN_SCRATCH", "32768"),
           os.environ.get("GCN_GBUFS", "2"), os.environ.get("GCN_SBUFS", "2"))
    if key not in _PROGRAM_CACHE:
        _PROGRAM_CACHE[key] = _build_program(t_key)
    return _PROGRAM_CACHE[key]


def _run(nc, in_maps, trace=False, **kw):
    from concourse.bass_utils import run_bass_kernel_spmd

    return run_bass_kernel_spmd(nc, in_maps, core_ids=list(range(NCORES)),
                                trace=trace, **kw)


def kernel(X, weight, degrees, row_pointers, column_index, _trace=False, _ret_raw=False):
    assert X.shape == (N, F) and column_index.shape == (E,)
    xt, w16, idx_w, dst_t, degt, dest_of, t_key = _preprocess(
        X, weight, degrees, row_pointers, column_index
    )
    nc = _get_program(t_key)
    g0cols = min(GB_TILES * P // 16, idx_w[0].shape[2])
    in_maps = []
    for c in range(NCORES):
        m = {"xt": xt, "degt": degt[c], "w16": w16,
             "idx0blk": np.ascontiguousarray(
                 np.concatenate([idx_w[q][c][:, :g0cols] for q in range(NQ)], axis=1))}
        if idx_w[0].shape[2] > g0cols:
            m["idxrest"] = np.ascontiguousarray(
                np.concatenate([idx_w[q][c][:, g0cols:] for q in range(NQ)], axis=1))
        for q in range(NQ):
            m[f"dstrel{q}"] = dst_t[q][c]
        in_maps.append(m)
    res = _run(nc, in_maps, trace=_trace)
    out = np.empty((N, F), np.float32)
    for c in range(NCORES):
        # device layout: partition s = (w%2)*W + pos, free block pw = w//2
        r = res.results[c]["out"].reshape(2, W, NPAIR, F)
        rw = r.transpose(2, 0, 1, 3).reshape(NW * W, F)      # [(w, pos), F]
        ids = dest_of[c]
        valid = ids >= 0
        out[ids[valid]] = rw[valid]
    if _ret_raw:
        return out, res
    return out



# revision 36
# speedup vs baseline: 1.1611x; 1.1611x over previous
"""GCN forward (gather + segment-sum + matmul) on 8 TRN2 NeuronCores.

Algorithm (factorized GCN):
    out[i] = deg[i] * (sum_{j in N(i)} deg[j] * X[j]) @ W

Sharding: destination nodes are split across the 8 cores (12500 rows each);
the deg_src-prescaled fp16 feature table X' = deg[:,None]*X is replicated to
every core's HBM. Each core:
  - bin-packs its 12500 dests into 200 windows of <=64 (64-wide one-hots
    halve the DVE sel build and the per-edge-tile matmul stream vs 128;
    window pairs share one 128-slot output block), every (chunk,window)
    cell holds <=256 edges = exactly 2 gather tiles (~1.4% tile padding),
  - gathers the fp16 rows of X' for its ~200K edges with gpsimd dma_gather
    on 4 SWDGE queues (the bottleneck: Q7 desc-gen is ~90% busy and the HW
    activity throttle holds utilization near 0.5; int16 gather indices
    force a 4-way chunking of the 100K-row table, so each core keeps 4
    chunk-local edge streams, each cell's edges sorted by source),
  - the 4 streams are padded to EQUAL tile counts and calls are emitted in
    strict q0->q3 rotation with all first-call indices landed by one DMA:
    the Tile scheduler assigns SWDGE completion sems round-robin over 8
    lanes ignoring queue_num, and only a periodic issue order keeps each
    lane locked to one queue (violations corrupt gather completions),
  - builds one-hot matrices sel[e,d] = (dstrel[e] == d) in batches of SB_T
    tiles with a single broadcast-AP DVE is_equal,
  - segment-sums via TensorE: A_T[f,d] += G[e,f]^T @ sel[e,d], accumulating
    in PSUM over a window's edge tiles round-robined across the 4 chunk
    streams; window pairs then share the @W matmul and deg_dest scaling,
  - flushes eighths of the persistent SBUF output buffer as they finish,
  - the host inverse-permutes the rows (window packing) and concatenates.

The per-edge aggregation, both matmuls and the deg_dest scaling run on
device; the host computes indices/partitioning and stages dtype-converted,
deg_src-prescaled inputs.
"""
import os

import numpy as np

N = 100000
E = 1600000
F = 128
P = 128
NCORES = 8
NPC = N // NCORES          # 12500 destination rows per core
# Destination windows are W=64 wide (halving the one-hot sel width and the
# per-edge-tile matmul stream from 128 to 64 columns cuts DVE and TensorE
# activity ~2x, easing the HW activity throttle). Two windows pair up into
# one 128-slot output block so the W@weight matmul / outbuf layout keeps
# full 128-partition width.
W = 64
NW = 200
NPAIR = NW // 2
NQ = 4                     # table chunks (int16 gather indices)
CHUNK = 25000              # rows per chunk
# Tiles per gather call. Per-call num_idxs is capped by the SWDGE
# descriptor-ring carveout, which scales with dynamic_dma_scratch_size:
# ~97 descs/lane at the 16KB default (13 tiles crashed the device), ~194
# at 32KB (20 tiles = 161 descs/lane verified on HW). Fewer, bigger calls
# amortize the ~1us fixed Q7 desc-gen cost per call.
_gbt_env = os.environ.get("GCN_GB_TILES", "20")
GBT = [int(x) for x in (_gbt_env.split(",") * 4)[:4]] if "," in _gbt_env \
    else [int(_gbt_env)] * 4
GB_TILES = max(GBT)

_PROGRAM_CACHE: dict = {}


def _row_ids_from_pointers(row_pointers: np.ndarray) -> np.ndarray:
    """Replicates jnp.repeat(arange(N), diff(rp), total_repeat_length=E)."""
    rl = np.diff(row_pointers.astype(np.int64))
    starts = np.concatenate([np.zeros(1, np.int64), np.cumsum(rl)[:-1]])
    return np.searchsorted(starts, np.arange(E, dtype=np.int64), side="right") - 1


def _group_dests(cnt):
    """Bin-pack one core's destinations into NW windows of <=W dests.

    cnt: [NPC, NQ] per-dest per-chunk edge counts. Windows 2..NW-1 are
    hard-capped at 2*P edges per chunk (exactly 2 gather tiles); windows
    0-1 absorb the overflow. Any grouping is correct (t_qw is computed
    from actual counts); the caps only minimize tile padding.
    Returns (grp [NPC], pos [NPC]).
    """
    CAP = 4 * P * W // 128
    NREG = NW - 2
    loads = np.zeros((NREG, NQ), np.int64)
    sizes = np.zeros(NREG, np.int64)
    grp = np.full(NPC, -1, np.int64)
    order = np.argsort(-cnt.max(axis=1), kind="stable")
    overflow = []
    big = 1 << 40
    for d in order:
        v = cnt[d]
        cand = loads + v
        score = cand.max(axis=1)
        score[(cand > CAP).any(axis=1) | (sizes >= W)] = big
        g = int(np.argmin(score))
        if score[g] >= big:
            overflow.append(d)
            continue
        loads[g] = cand[g]
        sizes[g] += 1
        grp[d] = g + 2
    # overflow windows 0/1: size-capped only
    osz = [0, 0]
    for d in overflow:
        g = 0 if osz[0] <= osz[1] and osz[0] < W else 1
        if osz[g] >= W:  # both full: spill into least-loaded regular window
            g2 = int(np.argmin(np.where(sizes < W, loads.max(axis=1), big)))
            loads[g2] += cnt[d]
            sizes[g2] += 1
            grp[d] = g2 + 2
            continue
        osz[g] += 1
        grp[d] = g
    # positions within each window
    pos = np.zeros(NPC, np.int64)
    nxt = np.zeros(NW, np.int64)
    for d in range(NPC):
        g = grp[d]
        pos[d] = nxt[g]
        nxt[g] += 1
    return grp, pos


def _preprocess(X, weight, degrees, row_pointers, column_index):
    row_ids = _row_ids_from_pointers(row_pointers)          # [E] sorted, in [0,N)
    col = column_index.astype(np.int64)
    deg = np.ascontiguousarray(degrees.astype(np.float32))

    core = row_ids // NPC                                   # [E] in [0,8)
    local = row_ids - core * NPC
    q = col // CHUNK                                        # [E] in [0,4)
    src16_all = (col - q * CHUNK).astype(np.int16)

    # balanced dest->window grouping per core (kills tile padding)
    w_local = np.empty(E, np.int64)
    dstrel_all = np.empty(E, np.float32)
    dest_of = np.full((NCORES, NW * W), -1, np.int64)       # (c, w*W+p) -> node id
    for c in range(NCORES):
        m = core == c
        cnt = np.bincount(local[m] * NQ + q[m], minlength=NPC * NQ).reshape(NPC, NQ)
        grp, pos = _group_dests(cnt)
        w_local[m] = grp[local[m]]
        dstrel_all[m] = pos[local[m]]
        dest_of[c, grp * W + pos] = c * NPC + np.arange(NPC)

    key = ((core * NQ + q) * NW + w_local).astype(np.int64)  # (c, q, w)
    counts = np.bincount(key, minlength=NCORES * NQ * NW).reshape(NCORES, NQ, NW)
    t_qw = -(-counts.max(axis=0) // P)                       # [NQ, NW]
    # Equalize the 4 chunk streams to the same tile count (pad tiles have
    # src=0, dstrel=-1 and contribute nothing): equal streams mean every
    # queue issues the same number of gather calls, so the strict
    # q0->q3 call rotation in the device program stays exactly periodic.
    # The Tile scheduler assigns SWDGE completion sems round-robin over 8
    # lanes ignoring queue_num; only a periodic issue order keeps each
    # lane pinned to one queue (cross-queue increments desync the ucode's
    # per-queue reclaim and corrupt the gather data).
    T_tiles = int(t_qw.sum(axis=1).max())
    for qq in range(NQ):
        t_qw[qq, 0] += T_tiles - int(t_qw[qq].sum())
    lq = t_qw.sum(axis=1) * P                                # [NQ] stream lengths
    chunk_base = np.concatenate([np.zeros(1, np.int64), np.cumsum(lq)])
    ltot = int(chunk_base[-1])
    # offset of window w's padded segment within chunk q's stream
    offs_qw = np.cumsum(np.concatenate([np.zeros((NQ, 1), np.int64), t_qw[:, :-1]], axis=1) * P, axis=1) \
        if False else (np.cumsum(t_qw, axis=1) - t_qw) * P   # [NQ, NW] exclusive prefix

    # within each (core,chunk,window) cell, order edges by ascending source so
    # each SDMA engine's gather descriptors walk HBM mostly monotonically
    order = np.lexsort((src16_all, key))
    key_s = key[order]
    starts_flat = np.concatenate([np.zeros(1, np.int64), np.cumsum(counts.reshape(-1))])[:-1]
    rank_s = np.arange(E, dtype=np.int64) - starts_flat[key_s]
    q_s = (key_s // NW) % NQ
    w_s = key_s % NW
    core_s = key_s // (NQ * NW)
    pos_s = chunk_base[q_s] + offs_qw[q_s, w_s] + rank_s     # [E] position in core's array

    src_pad = np.zeros((NCORES, ltot), np.int16)
    dstrel_pad = np.full((NCORES, ltot), -1.0, np.float32)
    src_pad[core_s, pos_s] = src16_all[order]
    dstrel_pad[core_s, pos_s] = dstrel_all[order]

    # per-chunk device layouts
    idx_w, dst_t = [], []
    for qq in range(NQ):
        sl = slice(int(chunk_base[qq]), int(chunk_base[qq + 1]))
        s = src_pad[:, sl]                                   # [NC, LQ]
        # wrapped idx layout [128, LQ/16]: idx i at [i%16, i//16], replicated 8x
        iw = np.tile(s.reshape(NCORES, -1, 16).transpose(0, 2, 1), (1, 8, 1))
        idx_w.append(np.ascontiguousarray(iw))
        dst_t.append(np.ascontiguousarray(
            dstrel_pad[:, sl].reshape(NCORES, -1, P).transpose(0, 2, 1).astype(np.float16)))

    # per-core dest-degree table [P, NPAIR]: window pair pw = w//2 holds
    # window w's dests at partition s = (w%2)*W + pos
    degt = np.zeros((NCORES, P, NPAIR), np.float32)
    for c in range(NCORES):
        ids = dest_of[c]                                     # [NW*W]
        dv = np.where(ids >= 0, deg[np.clip(ids, 0, N - 1)], 0.0)
        degt[c] = dv.reshape(NPAIR, 2, W).transpose(1, 2, 0).reshape(P, NPAIR)

    # stage deg_src-prescaled features: the weighted segment-sum's per-edge
    # weights deg[col] fold into the gathered rows (host staging, like the
    # dtype conversion); the aggregation itself stays on device
    xt = np.ascontiguousarray((X.astype(np.float32) * deg[:, None]).astype(np.float16))
    w16 = np.ascontiguousarray(weight.astype(np.float16))
    t_key = tuple(tuple(int(x) for x in row) for row in t_qw)
    return xt, w16, idx_w, dst_t, degt, dest_of, t_key


SB_T = int(os.environ.get("GCN_SB_T", "8"))  # tiles per batched sel build


def _build_program(t_qw):
    import concourse.bacc as bacc
    import concourse.bass as bass
    import concourse.mybir as mybir
    import concourse.tile as tile

    lq = [sum(t_qw[q]) * P for q in range(NQ)]

    scratch = int(os.environ.get("GCN_SCRATCH", "32768"))
    # GCN_QUEUES=2 maps the 4 chunk streams onto 2 SWDGE queues
    # (queue = chunk % 2): if the descriptor-ring carveout is split per
    # queue, each ring doubles, allowing twice-as-big calls
    # (GCN_GB_TILES=40) and halving the per-call fixed Q7 cost. Chunk
    # emission rotation 0,1,2,3 -> queue 0,1,0,1 stays periodic, so
    # DMASW lane locks remain queue-consistent.
    NQUEUES = int(os.environ.get("GCN_QUEUES", "4"))
    nc = bacc.Bacc("TRN2", target_bir_lowering=False, num_swdge_queues=NQUEUES,
                   dynamic_dma_scratch_size=scratch)
    xt_p = nc.declare_dram_parameter("xt", [N, F], mybir.dt.float16, isOutput=False)
    # All 4 queues' gather indices ride TWO shared parameters, each loaded
    # by ONE DMA: a small "first block" slab (gates every queue's first
    # gather, lands in ~1us) and the rest. One DMA per slab keeps every
    # queue's next call becoming ready at the same instant, so the tile
    # scheduler's issue order (and thus its round-robin DMASW lane
    # assignment) stays in strict q0->q3 rotation; staggered per-queue
    # loads would let q0 run ahead and cross-queue lane locks corrupt the
    # gather completion tracking.
    idx_step = lq[0] // 16
    assert all(l // 16 == idx_step for l in lq)
    g0cols = min(GBT[0] * P // 16, idx_step)
    idx0_p = nc.declare_dram_parameter("idx0blk", [P, NQ * g0cols], mybir.dt.int16, isOutput=False)
    idxrest_p = nc.declare_dram_parameter("idxrest", [P, NQ * (idx_step - g0cols)], mybir.dt.int16, isOutput=False) \
        if idx_step > g0cols else None
    dst_ps = [nc.declare_dram_parameter(f"dstrel{q}", [P, lq[q] // P], mybir.dt.float16, isOutput=False) for q in range(NQ)]
    degt_p = nc.declare_dram_parameter("degt", [P, NPAIR], mybir.dt.float32, isOutput=False)
    w_p = nc.declare_dram_parameter("w16", [F, F], mybir.dt.float16, isOutput=False)
    # transposed output layout: row s holds pair-major features so the
    # final DMA is one contiguous 49KB-per-partition write (host unshuffles)
    out_p = nc.declare_dram_parameter("out", [P, NPAIR * F], mybir.dt.float32, isOutput=True)

    def bcast_mid(ap, t):
        # [128, t] AP -> [128, t, W] with stride-0 inner (value per (p, tile))
        return bass.AP(ap.tensor, ap.offset, [ap.ap[0], [ap.ap[1][0], t], [0, W]])

    # prep/trigger split measured 5x SLOWER (per-call trigger+sem overhead in
    # Tile mode swamps the gen/drain overlap it buys); keep the blocking form
    use_prep = os.environ.get("GCN_PREP", "0") == "1"
    dma_sems = [nc.alloc_semaphore(f"swdge_dma{q}") for q in range(NQ)] if use_prep else None
    with tile.TileContext(nc) as tc:
        with (
            tc.tile_pool(name="persist", bufs=1) as persist,
            tc.tile_pool(name="gblk", bufs=int(os.environ.get("GCN_GBUFS", "2"))) as gpool,
            tc.tile_pool(name="selp", bufs=int(os.environ.get("GCN_SBUFS", "2"))) as selpool,
            tc.tile_pool(name="atsb", bufs=2) as atpool,
            tc.tile_pool(name="outsb", bufs=2) as outpool,
            tc.tile_pool(name="psum1", bufs=2, space="PSUM") as psum1,
            tc.tile_pool(name="psum2", bufs=2, space="PSUM") as psum2,
        ):
            # (Tried issuing the small loads on the Scalar HWDGE ring so
            # they don't queue behind the 3MB idxrest transfer: the first
            # gather only moved 24.5->22.8us — the ~21us head is a
            # scheduler stage-reset barrier waiting on ALL stage-0 DMAs
            # including idxrest, so the ring choice is neutral.)
            idx0_sb = persist.tile([P, NQ * g0cols], mybir.dt.int16, tag="idx0blk", name="idx0blk")
            nc.sync.dma_start(idx0_sb[:], idx0_p[:])
            rest_cols = idx_step - g0cols
            if idxrest_p is not None:
                idxr_sb = persist.tile([P, NQ * rest_cols], mybir.dt.int16, tag="idxrest", name="idxrest")
                nc.sync.dma_start(idxr_sb[:], idxrest_p[:])
            dst_sb = []
            for q in range(NQ):
                t2 = persist.tile([P, lq[q] // P], mybir.dt.float16, tag=f"dst{q}", name=f"dst{q}")
                nc.sync.dma_start(t2[:], dst_ps[q][:])
                dst_sb.append(t2)
            degt_sb = persist.tile([P, NPAIR], mybir.dt.float32)
            nc.sync.dma_start(degt_sb[:], degt_p[:])
            w_sb = persist.tile([F, F], mybir.dt.float16)
            nc.sync.dma_start(w_sb[:], w_p[:])
            c_i32 = persist.tile([P, P], mybir.dt.int32)
            nc.gpsimd.iota(c_i32[:], pattern=[[1, P]], base=0, channel_multiplier=0)
            c_f16 = persist.tile([P, P], mybir.dt.float16)
            nc.vector.tensor_copy(c_f16[:], c_i32[:])

            outbuf = persist.tile([P, NPAIR * F], mybir.dt.float32, name="outbuf")

            pos = [0] * NQ
            calls_done = [0] * NQ
            gblk = [None] * NQ
            selblk = [None] * NQ
            # strict round-robin gather-call emission: all 4 streams have
            # equal tile counts (t_qw equalized in _preprocess) and every
            # call is GBT tiles, so the SWDGE instruction order is exactly
            # periodic q0,q1,q2,q3 and the scheduler's round-robin DMASW
            # lane assignment keeps each lane locked to a single queue.
            T_tiles = lq[0] // P
            assert all(l // P == T_tiles for l in lq)
            GBT_U = GBT[0]
            covered = [0] * NQ           # tiles covered by emitted calls
            blocks = [[] for _ in range(NQ)]   # FIFO of (start, end, tile, call#)
            cur_start = [0] * NQ
            cur_end = [0] * NQ
            cur_call = [0] * NQ
            rot = [0]

            def emit_one(qe):
                start = covered[qe]
                nt_call = min(GBT_U, T_tiles - start)
                if nt_call <= 0:
                    return
                nidx = nt_call * P
                t = gpool.tile([P, GBT_U * F], mybir.dt.float16,
                               tag=f"gblk{qe}", name=f"gblk{qe}")
                c0 = start * P // 16
                c1 = (start * P + nidx) // 16
                if start == 0:
                    assert c1 <= g0cols
                    iap = idx0_sb[:, qe * g0cols : qe * g0cols + c1]
                else:
                    iap = idxr_sb[:, qe * rest_cols + (c0 - g0cols)
                                  : qe * rest_cols + (c1 - g0cols)]
                gather_kw = dict(
                    out_ap=t[:, : nt_call * F].rearrange("p (k f) -> p k f", f=F),
                    in_ap=xt_p[qe * CHUNK : (qe + 1) * CHUNK, :],
                    idxs_ap=iap,
                    num_idxs=nidx,
                    num_idxs_reg=nidx,
                    elem_size=F,
                    queue_num=qe % NQUEUES,
                    single_packet=(os.environ.get('GCN_SP', '0') == '1'),
                )
                if use_prep:
                    nc.gpsimd.dma_gather(
                        prepare_only=True, sem=dma_sems[qe], **gather_kw
                    )
                    nc.gpsimd.trigger_dma(count=None, queue_num=qe % NQUEUES)
                    calls_done[qe] += 1
                else:
                    nc.gpsimd.dma_gather(**gather_kw)
                blocks[qe].append((start, start + nt_call, t, calls_done[qe]))
                covered[qe] += nt_call

            flushed = 0
            for pw in range(NPAIR):
                wins = (2 * pw, 2 * pw + 1)
                nt_h = [sum(t_qw[q][w] for q in range(NQ)) for w in wins]
                if nt_h[0] + nt_h[1] == 0:
                    nc.vector.memset(outbuf[:, pw * F : (pw + 1) * F], 0.0)
                    continue
                at_sb = atpool.tile([F, P], mybir.dt.float16)
                for half, w in enumerate(wins):
                    if nt_h[half] == 0:
                        nc.vector.memset(at_sb[:, half * W : (half + 1) * W], 0.0)
                        continue
                    at_ps = psum1.tile([F, W], mybir.dt.float32, space="PSUM")
                    k = 0
                    # round-robin the window's tiles across the 4 chunk
                    # streams so the gather buffers free at an even pace
                    rr = [q for t in range(max(t_qw[q][w] for q in range(NQ)))
                          for q in range(NQ) if t < t_qw[q][w]]
                    if os.environ.get("GCN_RR", "1") != "1":
                        rr = [q for q in range(NQ) for _ in range(t_qw[q][w])]
                    for q in rr:
                        while pos[q] >= cur_end[q]:
                            while not blocks[q]:
                                emit_one(rot[0])
                                rot[0] = (rot[0] + 1) % NQ
                            (cur_start[q], cur_end[q], gblk[q],
                             cur_call[q]) = blocks[q].pop(0)
                        if pos[q] % SB_T == 0:
                            nt_s = min(SB_T, lq[q] // P - pos[q])
                            selblk[q] = selpool.tile(
                                [P, SB_T * W], mybir.dt.float16,
                                tag=f"sel{q}", name=f"sel{q}",
                            )
                            c_b = bass.AP(c_f16[:].tensor, c_f16[:].offset,
                                          [c_f16[:].ap[0], [0, nt_s], [1, W]])
                            nc.vector.tensor_tensor(
                                out=selblk[q][:, : nt_s * W].rearrange("p (t w) -> p t w", w=W),
                                in0=c_b,
                                in1=bcast_mid(dst_sb[q][:, pos[q] : pos[q] + nt_s], nt_s),
                                op=mybir.AluOpType.is_equal,
                            )
                        j = pos[q] - cur_start[q]
                        js = pos[q] % SB_T
                        if use_prep and j == 0:
                            # gate the block's first consumer on DMA landed
                            nc.tensor.wait_ge(dma_sems[q], 16 * cur_call[q])
                        nc.tensor.matmul(
                            out=at_ps[:],
                            lhsT=gblk[q][:, j * F : (j + 1) * F],
                            rhs=selblk[q][:, js * W : (js + 1) * W],
                            start=(k == 0),
                            stop=(k == nt_h[half] - 1),
                        )
                        pos[q] += 1
                        k += 1
                    nc.scalar.activation(at_sb[:, half * W : (half + 1) * W],
                                         at_ps[:], mybir.ActivationFunctionType.Copy)
                o2_ps = psum2.tile([P, F], mybir.dt.float32, space="PSUM")
                nc.tensor.matmul(out=o2_ps[:], lhsT=at_sb[:], rhs=w_sb[:], start=True, stop=True)
                nc.scalar.activation(outbuf[:, pw * F : (pw + 1) * F], o2_ps[:],
                                     mybir.ActivationFunctionType.Copy,
                                     scale=degt_sb[:, pw : pw + 1])
                # flush finished eighths of the output buffer so the
                # writeback overlaps compute instead of trailing serially
                if (pw + 1) % (NPAIR // 8) == 0 and pw + 1 < NPAIR:
                    nc.sync.dma_start(out=out_p[:, flushed * F : (pw + 1) * F],
                                      in_=outbuf[:, flushed * F : (pw + 1) * F])
                    flushed = pw + 1
            nc.sync.dma_start(out=out_p[:, flushed * F :], in_=outbuf[:, flushed * F :])
    nc.compile()
    return nc


def _get_program(t_key):
    key = (t_key, tuple(GBT), SB_T, os.environ.get("GCN_QUEUES", "4"),
           os.environ.get("GCN_SCRATCH", "32768"),
           os.environ.get("GCN_GBUFS", "2"), os.environ.get("GCN_SBUFS", "2"))
    if key not in _PROGRAM_CACHE:
        _PROGRAM_CACHE[key] = _build_program(t_key)
    return _PROGRAM_CACHE[key]


def _run(nc, in_maps, trace=False, **kw):
    from concourse.bass_utils import run_bass_kernel_spmd

    return run_bass_kernel_spmd(nc, in_maps, core_ids=list(range(NCORES)),
                                trace=trace, **kw)


def kernel(X, weight, degrees, row_pointers, column_index, _trace=False, _ret_raw=False):
    assert X.shape == (N, F) and column_index.shape == (E,)
    xt, w16, idx_w, dst_t, degt, dest_of, t_key = _preprocess(
        X, weight, degrees, row_pointers, column_index
    )
    nc = _get_program(t_key)
    g0cols = min(GB_TILES * P // 16, idx_w[0].shape[2])
    in_maps = []
    for c in range(NCORES):
        m = {"xt": xt, "degt": degt[c], "w16": w16,
             "idx0blk": np.ascontiguousarray(
                 np.concatenate([idx_w[q][c][:, :g0cols] for q in range(NQ)], axis=1))}
        if idx_w[0].shape[2] > g0cols:
            m["idxrest"] = np.ascontiguousarray(
                np.concatenate([idx_w[q][c][:, g0cols:] for q in range(NQ)], axis=1))
        for q in range(NQ):
            m[f"dstrel{q}"] = dst_t[q][c]
        in_maps.append(m)
    res = _run(nc, in_maps, trace=_trace)
    out = np.empty((N, F), np.float32)
    for c in range(NCORES):
        # device layout: partition s = (w%2)*W + pos, free block pw = w//2
        r = res.results[c]["out"].reshape(2, W, NPAIR, F)
        rw = r.transpose(2, 0, 1, 3).reshape(NW * W, F)      # [(w, pos), F]
        ids = dest_of[c]
        valid = ids >= 0
        out[ids[valid]] = rw[valid]
    if _ret_raw:
        return out, res
    return out

